# revision 3
# baseline (speedup 1.0000x reference)
"""Chamfer distance kernel for Trainium2 (8 NeuronCores, SPMD).

Problem: B=16 batches of two 4096-point 3D clouds; cost =
  sum_b 0.5*(mean_n min_m d2[b,n,m] + mean_m min_n d2[b,n,m]).

Sharding: data-parallel over batch; each of the 8 cores handles 2
batches, one pass over each 4096x4096 pair matrix serving BOTH
reduction directions.

Device algorithm (per core, per batch):
  The PE computes d2[n,m] directly in PSUM as a K=15 bf16 matmul: the
  host splits every operand hi/lo into exactly-representable bf16
  pieces (coords 2-level, |p|^2 / |q|^2 3-level), so each partial
  product is exact in fp32 and d2 error is ~1e-5 absolute.
    lhsT rows: xh yh zh xl yl zl 1 1 1 sp1 sp2 sp3 (P cloud)
    rhs  rows: -2Xh -2Yh -2Zh -2Xl -2Yl -2Zl sq1 sq2 sq3 1 1 1 (Q)
  (pairing: cross terms drop only lo*lo ~ 1e-5.)

  Per 128-row n-tile, m in 2 halves of 2048 (PSUM ping-pong, 4 banks
  each):
  - Act drains PSUM once: Y = -d2 as f16 (copy with scale=-1); the
    negation lets every later min run as MAX so the cross-partition
    finale can use Pool's C-axis reduce (which supports max only).
  - DVE dir-2: colA = max(colA, Y)  (f16 2x mode, running column max)
  - DVE dir-1: fold chain max(Y_lo, Y_hi) 4096->256 + reduce -> rmax
    column per tile (f16 2x folds).
  Finale per batch: Pool C-axis max over colA -> [1,4096] colmax; DMA
  rmax [128,32] and colmax [4096] to HBM; host averages in fp64.

  Engines: DVE ~4.2us/tile (bottleneck), Act ~3.4us, PE ~2.5us, all
  static instructions (no hardware loops, no SWDGE).
"""

import sys

sys.path.insert(0, "/opt/trn_rl_repo")

from contextlib import ExitStack

import numpy as np

import concourse.bass as bass
import concourse.tile as tile
from concourse import bacc, mybir
from concourse.bass_utils import run_bass_kernel_spmd

B, N, D = 16, 4096, 3
NCORES = 8
BPC = B // NCORES      # batches per core
K = 15                 # contraction rows (hi/lo split, see module doc)
NT = N // 128          # 32 n-tiles per batch
NEGBIG = -60000.0
AVG_SCALE = 0.5
F32 = mybir.dt.float32
F16 = mybir.dt.float16
BF16 = mybir.dt.bfloat16
MAX = mybir.AluOpType.max
X = mybir.AxisListType.X
C = mybir.AxisListType.C

_NC = {}


def _build():
    nc = bacc.Bacc("TRN2", target_bir_lowering=False, debug=False)
    # lhsT / rhs rows for both local batches, batch-major on free axis
    sl = nc.dram_tensor("sl", [K, BPC * N], BF16, kind="ExternalInput").ap()
    rr = nc.dram_tensor("rr", [K, BPC * N], BF16, kind="ExternalInput").ap()
    # out: per batch, negated row-maxes [128,NT] then negated col-max [N]
    ro = nc.dram_tensor("ro", [BPC, 128, NT], F32, kind="ExternalOutput").ap()
    co = nc.dram_tensor("co", [BPC, N], F32, kind="ExternalOutput").ap()

    with tile.TileContext(nc) as tc, ExitStack() as ctx:
        sb = ctx.enter_context(tc.tile_pool(name="sb", bufs=1))
        ps = ctx.enter_context(tc.tile_pool(name="ps", bufs=1, space="PSUM"))
        SL = sb.tile([K, BPC * N], BF16, tag="SL")
        RR = sb.tile([K, BPC * N], BF16, tag="RR")
        nc.sync.dma_start(SL[:], sl)
        nc.sync.dma_start(RR[:], rr)

        pp = [ps.tile([128, 2048], F32, name=f"pp{i}", tag=f"pp{i}")
              for i in range(2)]
        NY = 3
        Y = [sb.tile([128, N], F16, name=f"Y{i}", tag=f"Y{i}")
             for i in range(NY)]
        colA = sb.tile([128, N], F16, tag="colA")
        F1 = sb.tile([128, 2048], F16, tag="F1")
        F2 = sb.tile([128, 1024], F16, tag="F2")
        F3 = sb.tile([128, 512], F16, tag="F3")
        F4 = sb.tile([128, 256], F16, tag="F4")
        rmax = sb.tile([128, NT], F32, tag="rmax")
        cmax = sb.tile([1, N], F32, tag="cmax")

        for bl in range(BPC):
            b0 = bl * N
            for t in range(NT):
                dst = colA if t == 0 else Y[t % NY]
                for h in range(2):
                    p = pp[(2 * t + h) % 2]
                    for mc in range(4):
                        o = h * 2048 + mc * 512
                        nc.tensor.matmul(
                            p[:, mc * 512:(mc + 1) * 512],
                            SL[:, b0 + t * 128:b0 + (t + 1) * 128],
                            RR[:, b0 + o:b0 + o + 512],
                            start=True, stop=True)
                    # stage negated d2 as f16 (frees PSUM)
                    nc.scalar.mul(dst[:, h * 2048:(h + 1) * 2048], p[:], -1.0)
                if t > 0:
                    nc.vector.tensor_tensor(colA[:], dst[:], colA[:], op=MAX)
                # dir-1: fold 4096->256, then reduce -> rmax col t
                nc.vector.tensor_tensor(F1[:], dst[:, 0:2048],
                                        dst[:, 2048:4096], op=MAX)
                nc.vector.tensor_tensor(F2[:], F1[:, 0:1024],
                                        F1[:, 1024:2048], op=MAX)
                nc.vector.tensor_tensor(F3[:], F2[:, 0:512],
                                        F2[:, 512:1024], op=MAX)
                nc.vector.tensor_tensor(F4[:], F3[:, 0:256],
                                        F3[:, 256:512], op=MAX)
                nc.vector.tensor_reduce(rmax[:, t:t + 1], F4[:], op=MAX,
                                        axis=X)
            nc.sync.dma_start(ro[bl], rmax[:])
            nc.gpsimd.tensor_reduce(cmax[:], colA[:], op=MAX, axis=C)
            nc.sync.dma_start(co[bl], cmax[:])

    nc.compile()
    return nc


def get_nc(mode=None):
    if "nc" not in _NC:
        _NC["nc"] = _build()
    return _NC["nc"]


def _bf16(x):
    u = np.asarray(x, np.float32).view(np.uint32)
    return ((u + 0x7FFF + ((u >> 16) & 1)) & 0xFFFF0000).view(np.float32)


def _prep_inputs(points1, points2, mode=None):
    """Full inputs -> per-core {"sl": [K,8192] bf16, "rr": ...} maps."""
    import ml_dtypes

    p1 = np.asarray(points1, np.float32)
    p2 = np.asarray(points2, np.float32)
    # hi/lo coordinate split  [B, N, 3]
    p1h = _bf16(p1)
    p1l = _bf16(p1 - p1h)
    p2h = _bf16(p2)
    p2l = _bf16(p2 - p2h)
    # squared norms of the EFFECTIVE (split) coords, 3-level split
    sq1 = ((p1h.astype(np.float64) + p1l) ** 2).sum(-1)   # [B, N]
    sq2 = ((p2h.astype(np.float64) + p2l) ** 2).sum(-1)

    def split3(v):
        a = _bf16(v.astype(np.float32))
        r = v - a.astype(np.float64)
        b = _bf16(r.astype(np.float32))
        r2 = r - b.astype(np.float64)
        c = _bf16(r2.astype(np.float32))
        return a, b, c

    s1a, s1b, s1c = split3(sq1)
    s2a, s2b, s2c = split3(sq2)

    maps = []
    for cid in range(NCORES):
        sl = np.zeros((K, BPC * N), np.float32)
        rr = np.zeros((K, BPC * N), np.float32)
        for bl in range(BPC):
            gb = cid * BPC + bl
            s = slice(bl * N, (bl + 1) * N)
            # k 0-2:  p1h . (-2 p2h)   k 3-5: p1h . (-2 p2l)
            # k 6-8:  p1l . (-2 p2h)   k 9-11: 1 . sq2{a,b,c}
            # k 12-14: sq1{a,b,c} . 1
            sl[0:3, s] = p1h[gb].T
            sl[3:6, s] = p1h[gb].T
            sl[6:9, s] = p1l[gb].T
            sl[9:12, s] = 1.0
            sl[12, s] = s1a[gb]
            sl[13, s] = s1b[gb]
            sl[14, s] = s1c[gb]
            rr[0:3, s] = -2.0 * p2h[gb].T
            rr[3:6, s] = -2.0 * p2l[gb].T
            rr[6:9, s] = -2.0 * p2h[gb].T
            rr[9, s] = s2a[gb]
            rr[10, s] = s2b[gb]
            rr[11, s] = s2c[gb]
            rr[12:15, s] = 1.0
        maps.append({
            "sl": sl.astype(ml_dtypes.bfloat16),
            "rr": rr.astype(ml_dtypes.bfloat16),
        })
    return maps


def _assemble(results):
    total = 0.0
    for cid in range(NCORES):
        r = results[cid]
        ro = r["ro"].astype(np.float64)   # [BPC, 128, NT] negated rowmins
        co = r["co"].astype(np.float64)   # [BPC, N] negated colmins
        for bl in range(BPC):
            m1 = -ro[bl].mean()
            m2 = -co[bl].mean()
            total += AVG_SCALE * (m1 + m2)
    return np.asarray(total, dtype=np.float32)


def run(points1, points2, trace=False, tmpdir=None, mode=None):
    nc = get_nc()
    in_maps = _prep_inputs(points1, points2)
    res = run_bass_kernel_spmd(nc, in_maps, list(range(NCORES)),
                               trace=trace, tmpdir=tmpdir)
    return _assemble(res.results), res


def kernel(points1, points2):
    out, _ = run(points1, points2)
    return out
